# revision 1
# baseline (speedup 1.0000x reference)
"""Trainium2 kernel for CompactBilinearLayer (count-sketch bilinear pooling).

Math: reference computes y = l2norm(signed_sqrt(sum_hw Re IFFT(FFT(x@M1)*FFT(x@M2)))).
Since M1/M2 are count-sketch matrices (one +-1 per row), FFT(x@M1) == x @ A1 with
A1[c,k] = s1[c] * exp(-2pi i h1[c] k / P) — a dense [512, K] matrix computable on the
host from M1 in O(C*K). The IFFT is linear, so the spatial sum moves before it.
Hermitian symmetry means only k = 0..4096 are needed.  Per core (4 batch elements,
784 spatial positions — fully batch-local, no collectives):
  A: P1/P2 projections  = A^T @ x^T     (split-fp32r: hi/lo RNE-11 parts, 3
     full-rate matmuls == exact fp32 quality at 3/4 the PE cost of native fp32)
  B: S[k,b] = sum_t (P1*P2) per batch   (complex product + segmented reduce)
  C: IFFT via two-step factorization n=64q+s: U/V twiddle (DVE) + matmul over k%128
  D: signed sqrt + per-batch L2 norm + store
"""
import numpy as np

P = 8192
C = 512
FT = 33            # frequency tiles of 128 -> 4224 slots >= 4097
NSLOT = FT * 128
NCORES = 8
BPC = 4            # batch elems per core
HW = 196           # spatial positions per batch elem
T = BPC * HW       # 784 positions per core
B = 32

_CACHE = {}


def _build_program():
    import concourse.bass as bass
    import concourse.tile as tile
    from concourse import bacc, mybir

    f32 = mybir.dt.float32
    f32r = mybir.dt.float32r
    nc = bacc.Bacc("TRN2", target_bir_lowering=False, debug=False,
                   num_devices=NCORES)

    ah_d = nc.dram_tensor("ah", [FT, C, 512], f32r, kind="ExternalInput").ap()
    al_d = nc.dram_tensor("al", [FT, C, 512], f32r, kind="ExternalInput").ap()
    xh_d = nc.dram_tensor("xh", [C, T], f32r, kind="ExternalInput").ap()
    xl_d = nc.dram_tensor("xl", [C, T], f32r, kind="ExternalInput").ap()
    cphi_d = nc.dram_tensor("cphi", [FT, 128, 64], f32, kind="ExternalInput").ap()
    sphi_d = nc.dram_tensor("sphi", [FT, 128, 64], f32, kind="ExternalInput").ap()
    cosa_d = nc.dram_tensor("cosa", [128, 128], f32, kind="ExternalInput").ap()
    nsina_d = nc.dram_tensor("nsina", [128, 128], f32, kind="ExternalInput").ap()
    y_d = nc.dram_tensor("y", [BPC, P], f32, kind="ExternalOutput").ap()

    mult = mybir.AluOpType.mult
    Act = mybir.ActivationFunctionType

    with tile.TileContext(nc) as tc:
        with (
            tc.tile_pool(name="const", bufs=1) as const,
            tc.tile_pool(name="apool", bufs=3) as apool,
            tc.tile_pool(name="ps", bufs=1, space="PSUM") as pspool,
            tc.tile_pool(name="scr", bufs=3) as scr,
            tc.tile_pool(name="uv", bufs=4) as uvpool,
        ):
            xh_sb = const.tile([128, 4, T], f32r)
            nc.sync.dma_start(xh_sb[:], xh_d.rearrange("(ck p) t -> p ck t", p=128))
            xl_sb = const.tile([128, 4, T], f32r)
            nc.sync.dma_start(xl_sb[:], xl_d.rearrange("(ck p) t -> p ck t", p=128))
            cphi_sb = const.tile([128, FT, 64], f32)
            nc.sync.dma_start(cphi_sb[:], cphi_d.rearrange("kt p s -> p kt s"))
            sphi_sb = const.tile([128, FT, 64], f32)
            nc.sync.dma_start(sphi_sb[:], sphi_d.rearrange("kt p s -> p kt s"))
            cosa_sb = const.tile([128, 128], f32)
            nc.sync.dma_start(cosa_sb[:], cosa_d)
            nsina_sb = const.tile([128, 128], f32)
            nc.sync.dma_start(nsina_sb[:], nsina_d)
            ones_sb = const.tile([128, 1], f32)
            nc.vector.memset(ones_sb[:], 1.0)
            sre_sb = const.tile([128, FT * 4], f32)
            sim_sb = const.tile([128, FT * 4], f32)

            # ---- stage A+B: projections, complex product, spatial reduce ----
            for ft in range(FT):
                ah_t = apool.tile([128, 4, 512], f32r, tag="ah")
                nc.sync.dma_start(
                    ah_t[:], ah_d[ft].rearrange("(ck p) m -> p ck m", p=128)
                )
                al_t = apool.tile([128, 4, 512], f32r, tag="al")
                nc.sync.dma_start(
                    al_t[:], al_d[ft].rearrange("(ck p) m -> p ck m", p=128)
                )
                ps = [
                    pspool.tile([128, T], f32, tag=f"p{m}", name=f"ps{m}_{ft}")
                    for m in range(4)
                ]
                for m in range(4):
                    msl = slice(m * 128, (m + 1) * 128)
                    for ck in range(4):
                        for c0, cn in ((0, 512), (512, T - 512)):
                            terms = (
                                (ah_t[:, ck, msl], xh_sb[:, ck, c0:c0 + cn]),
                                (ah_t[:, ck, msl], xl_sb[:, ck, c0:c0 + cn]),
                                (al_t[:, ck, msl], xh_sb[:, ck, c0:c0 + cn]),
                            )
                            for ti, (lhs, rhs) in enumerate(terms):
                                nc.tensor.matmul(
                                    ps[m][:, c0:c0 + cn],
                                    lhs,
                                    rhs,
                                    start=(ck == 0 and ti == 0),
                                    stop=(ck == 3 and ti == 2),
                                )
                # DVE reads at most one PSUM operand; stage the A2 pair in SBUF
                p2sb = scr.tile([128, T], f32, tag="p2sb")
                p3sb = scr.tile([128, T], f32, tag="p3sb")
                nc.scalar.activation(p2sb[:], ps[2][:], Act.Copy)
                nc.scalar.activation(p3sb[:], ps[3][:], Act.Copy)
                operands = ((ps[0], p2sb), (ps[1], p3sb), (ps[0], p3sb), (ps[1], p2sb))
                red = []
                for i, (pa, pb) in enumerate(operands):
                    prod = scr.tile([128, T], f32, tag=f"prod{i}",
                                    name=f"prod{i}_{ft}")
                    nc.vector.tensor_tensor(prod[:], pa[:], pb[:], op=mult)
                    r = scr.tile([128, BPC], f32, tag=f"red{i}",
                                 name=f"red{i}_{ft}")
                    nc.vector.reduce_sum(
                        out=r[:],
                        in_=prod[:].rearrange("p (b t) -> p b t", b=BPC),
                        axis=mybir.AxisListType.X,
                    )
                    red.append(r)
                sblk = slice(ft * 4, (ft + 1) * 4)
                nc.vector.tensor_sub(sre_sb[:, sblk], red[0][:], red[1][:])
                nc.vector.tensor_add(sim_sb[:, sblk], red[2][:], red[3][:])

            # ---- stage C: twiddle + IFFT matmul over k mod 128 ----
            psy = pspool.tile([128, BPC * 64], f32, tag="p0")
            for kt in range(FT):
                cph = cphi_sb[:, kt, :][:, None, :].broadcast_to([128, BPC, 64])
                sph = sphi_sb[:, kt, :][:, None, :].broadcast_to([128, BPC, 64])
                sre = sre_sb[:, kt * 4:(kt + 1) * 4][:, :, None].broadcast_to(
                    [128, BPC, 64])
                sim = sim_sb[:, kt * 4:(kt + 1) * 4][:, :, None].broadcast_to(
                    [128, BPC, 64])
                u1 = uvpool.tile([128, BPC, 64], f32, tag="u1")
                u2 = uvpool.tile([128, BPC, 64], f32, tag="u2")
                uu = uvpool.tile([128, BPC * 64], f32, tag="uu")
                v1 = uvpool.tile([128, BPC, 64], f32, tag="v1")
                v2 = uvpool.tile([128, BPC, 64], f32, tag="v2")
                vv = uvpool.tile([128, BPC * 64], f32, tag="vv")
                nc.vector.tensor_tensor(u1[:], cph, sre, op=mult)
                nc.vector.tensor_tensor(u2[:], sph, sim, op=mult)
                nc.vector.tensor_sub(
                    uu[:].rearrange("p (b s) -> p b s", b=BPC), u1[:], u2[:])
                nc.vector.tensor_tensor(v1[:], sph, sre, op=mult)
                nc.vector.tensor_tensor(v2[:], cph, sim, op=mult)
                nc.vector.tensor_add(
                    vv[:].rearrange("p (b s) -> p b s", b=BPC), v1[:], v2[:])
                nc.tensor.matmul(psy[:], cosa_sb[:], uu[:],
                                 start=(kt == 0), stop=False)
                nc.tensor.matmul(psy[:], nsina_sb[:], vv[:],
                                 start=False, stop=(kt == FT - 1))

            # ---- stage D: signed sqrt, per-batch l2 norm, store ----
            absy = scr.tile([128, BPC * 64], f32, tag="absy")
            nc.scalar.activation(absy[:], psy[:], Act.Abs)
            sqy = scr.tile([128, BPC * 64], f32, tag="sqy")
            nc.scalar.activation(sqy[:], absy[:], Act.Sqrt)
            sgn = scr.tile([128, BPC * 64], f32, tag="sgn")
            nc.scalar.activation(sgn[:], psy[:], Act.Sign)
            ys = scr.tile([128, BPC * 64], f32, tag="ys")
            nc.vector.tensor_mul(ys[:], sqy[:], sgn[:])

            psn = pspool.tile([128, BPC * 64], f32, tag="p1")
            nc.tensor.matmul(psn[0:1, :], ones_sb[:], absy[:],
                             start=True, stop=True)
            nsq = scr.tile([1, BPC], f32, tag="nsq")
            nc.vector.reduce_sum(
                out=nsq[:],
                in_=psn[0:1, :].rearrange("p (b s) -> p b s", b=BPC),
                axis=mybir.AxisListType.X,
            )
            nc.vector.tensor_scalar_max(nsq[:], nsq[:], 1e-10)
            sqn = scr.tile([1, BPC], f32, tag="sqn")
            nc.scalar.activation(sqn[:], nsq[:], Act.Sqrt)
            invn = scr.tile([1, BPC], f32, tag="invn")
            nc.vector.reciprocal(invn[:], sqn[:])

            onesrow = const.tile([1, 128], f32)
            nc.vector.memset(onesrow[:], 1.0)
            psb = pspool.tile([128, BPC * 64], f32, tag="p2")
            nc.tensor.matmul(psb[:, 0:BPC], onesrow[0:1, :], invn[0:1, :],
                             start=True, stop=True)
            inv_b = psb[:, 0:BPC][:, :, None].broadcast_to([128, BPC, 64])
            fin = scr.tile([128, BPC * 64], f32, tag="fin")
            nc.vector.tensor_tensor(
                fin[:].rearrange("p (b s) -> p b s", b=BPC),
                ys[:].rearrange("p (b s) -> p b s", b=BPC),
                inv_b,
                op=mult,
            )
            for b in range(BPC):
                nc.sync.dma_start(
                    y_d[b].rearrange("(q s) -> q s", q=128),
                    fin[:, b * 64:(b + 1) * 64],
                )

    nc.compile()
    return nc


def _round_fp32r(f):
    """RNE to 11 mantissa bits — matches TRN2 fp32r rounding exactly."""
    u = np.ascontiguousarray(f).view(np.uint32)
    drop = 12
    r = u + np.uint32((1 << (drop - 1)) - 1) + ((u >> drop) & np.uint32(1))
    r = (r >> drop) << drop
    return r.view(np.float32)


def _split_fp32r(f):
    hi = _round_fp32r(f)
    lo = _round_fp32r((f - hi).astype(np.float32))
    return hi, lo


def _host_prep(x, M1, M2):
    x = np.ascontiguousarray(np.asarray(x, np.float32))
    M1 = np.asarray(M1, np.float32)
    M2 = np.asarray(M2, np.float32)

    h1 = np.argmax(np.abs(M1), axis=1)
    s1 = M1[np.arange(C), h1].astype(np.float64)
    h2 = np.argmax(np.abs(M2), axis=1)
    s2 = M2[np.arange(C), h2].astype(np.float64)

    k = np.arange(NSLOT, dtype=np.float64)
    valid = k <= P // 2
    ang1 = 2 * np.pi * np.outer(h1.astype(np.float64), k) / P
    ang2 = 2 * np.pi * np.outer(h2.astype(np.float64), k) / P
    # a[ft, c, m*128 + j]: m in (A1re, A1im, A2re, A2im), freq = ft*128 + j
    a = np.empty((FT, C, 512), np.float32)
    a1re = (s1[:, None] * np.cos(ang1) * valid).astype(np.float32)
    a1im = (-s1[:, None] * np.sin(ang1) * valid).astype(np.float32)
    a2re = (s2[:, None] * np.cos(ang2) * valid).astype(np.float32)
    a2im = (-s2[:, None] * np.sin(ang2) * valid).astype(np.float32)
    for ft in range(FT):
        ksl = slice(ft * 128, (ft + 1) * 128)
        a[ft, :, 0:128] = a1re[:, ksl]
        a[ft, :, 128:256] = a1im[:, ksl]
        a[ft, :, 256:384] = a2re[:, ksl]
        a[ft, :, 384:512] = a2im[:, ksl]

    w = np.where(valid, 2.0 / P, 0.0)
    w[0] = 1.0 / P
    w[P // 2] = 1.0 / P
    s_idx = np.arange(64, dtype=np.float64)
    phi = 2 * np.pi * np.outer(k, s_idx) / P
    cphi = (w[:, None] * np.cos(phi)).astype(np.float32).reshape(FT, 128, 64)
    sphi = (w[:, None] * np.sin(phi)).astype(np.float32).reshape(FT, 128, 64)

    km = np.arange(128, dtype=np.float64)
    alpha = 2 * np.pi * np.outer(km, km) / 128
    cosa = np.cos(alpha).astype(np.float32)
    nsina = (-np.sin(alpha)).astype(np.float32)

    xt = np.ascontiguousarray(x.reshape(B * HW, C).T)  # [C, 6272]

    ah, al = _split_fp32r(a)
    xh, xl = _split_fp32r(xt)
    return ah, al, cphi, sphi, cosa, nsina, xh, xl


def _make_in_maps(x, M1, M2):
    ah, al, cphi, sphi, cosa, nsina, xh, xl = _host_prep(x, M1, M2)
    in_maps = []
    for r in range(NCORES):
        in_maps.append({
            "ah": ah,
            "al": al,
            "xh": np.ascontiguousarray(xh[:, r * T:(r + 1) * T]),
            "xl": np.ascontiguousarray(xl[:, r * T:(r + 1) * T]),
            "cphi": cphi,
            "sphi": sphi,
            "cosa": cosa,
            "nsina": nsina,
        })
    return in_maps


def kernel(x, M1, M2):
    from concourse.bass_utils import run_bass_kernel_spmd

    if "nc" not in _CACHE:
        _CACHE["nc"] = _build_program()
    nc = _CACHE["nc"]

    in_maps = _make_in_maps(x, M1, M2)
    res = run_bass_kernel_spmd(nc, in_maps, core_ids=list(range(NCORES)))
    out = np.concatenate([res.results[r]["y"] for r in range(NCORES)], axis=0)
    return out.astype(np.float32)



# revision 11
# speedup vs baseline: 2.2019x; 2.2019x over previous
"""Trainium2 kernel for CompactBilinearLayer (count-sketch bilinear pooling).

Math: y = l2norm(signed_sqrt(sum_hw Re IFFT(FFT(x@M1)*FFT(x@M2)))).
FFT(x@M1) == x @ A1 with A1[c,k] = s1[c] exp(-2pi i h1[c] k/P) (dense [C,K],
host-built).  IFFT is linear so the spatial sum moves before it; Hermitian
symmetry keeps only k = 0..4096 (padded to 33*128 slots).

Per core (4 batch elems, T=784 spatial positions, no collectives):
  A: P-planes = A^T @ x^T in bf16 (tolerance 2e-2 >> bf16 error) as two
     2-plane PSUM super-tiles (re1,im1) and (re2,-im2).
  B: casts to bf16 SBUF (with an extra negated im2 plane so both complex
     product groups are pure ADDs), pair-packed DVE products, bf16 pair-fold,
     one segmented reduce -> S[k, b] (re, im).
  C: per kt twiddle U=cphi*Sre-sphi*Sim, V=sphi*Sre+cphi*Sim as packed
     TTs (GpSimd+DVE), accumulated over kt into Utot/Vtot; since the DFT-128
     matrix depends only on k mod 128, IFFT = 2 matmuls at the end.
  D: signed sqrt + per-batch L2 norm + store.
"""
import numpy as np

P = 8192
C = 512
FT = 33            # frequency tiles of 128 -> 4224 slots >= 4097
NCORES = 8
BPC = 4            # batch elems per core
HW = 196           # spatial positions per batch elem
T = BPC * HW       # 784 positions per core
B = 32

_CACHE = {}


def _build_program():
    import concourse.bass as bass
    import concourse.tile as tile
    from concourse import bacc, mybir

    f32 = mybir.dt.float32
    f16 = mybir.dt.float16
    nc = bacc.Bacc("TRN2", target_bir_lowering=False, debug=False,
                   num_devices=NCORES)

    a_d = nc.dram_tensor("a", [FT, C, 512], f16, kind="ExternalInput").ap()
    x_d = nc.dram_tensor("x", [C, T], f16, kind="ExternalInput").ap()
    cu_d = nc.dram_tensor("cu", [FT, 128, 2, 64], f32, kind="ExternalInput").ap()
    cv_d = nc.dram_tensor("cv", [FT, 128, 2, 64], f32, kind="ExternalInput").ap()
    cosa_d = nc.dram_tensor("cosa", [128, 128], f32, kind="ExternalInput").ap()
    nsina_d = nc.dram_tensor("nsina", [128, 128], f32, kind="ExternalInput").ap()
    y_d = nc.dram_tensor("y", [BPC, P], f32, kind="ExternalOutput").ap()

    mult = mybir.AluOpType.mult
    add = mybir.AluOpType.add
    Act = mybir.ActivationFunctionType

    with tile.TileContext(nc) as tc:
        with (
            tc.tile_pool(name="const", bufs=1) as const,
            tc.tile_pool(name="apool", bufs=3) as apool,
            tc.tile_pool(name="ps", bufs=1, space="PSUM") as pspool,
            tc.tile_pool(name="cast", bufs=2) as castp,
            tc.tile_pool(name="gp", bufs=2) as gpool,
            tc.tile_pool(name="uv", bufs=2) as uvpool,
            tc.tile_pool(name="scr", bufs=2) as scr,
        ):
            x_sb = const.tile([128, 4, T], f16)
            nc.sync.dma_start(x_sb[:], x_d.rearrange("(ck p) t -> p ck t", p=128))
            cu_sb = const.tile([128, FT, 2, 64], f32)
            nc.sync.dma_start(cu_sb[:], cu_d.rearrange("kt p c s -> p kt c s"))
            cv_sb = const.tile([128, FT, 2, 64], f32)
            nc.sync.dma_start(cv_sb[:], cv_d.rearrange("kt p c s -> p kt c s"))
            cosa_sb = const.tile([128, 128], f32)
            nc.sync.dma_start(cosa_sb[:], cosa_d)
            nsina_sb = const.tile([128, 128], f32)
            nc.sync.dma_start(nsina_sb[:], nsina_d)
            ones_sb = const.tile([128, 1], f32)
            nc.vector.memset(ones_sb[:], 1.0)
            onesrow = const.tile([1, 128], f32)
            nc.vector.memset(onesrow[:], 1.0)

            # S[k, group, b] (fp32): group 0 = Sre, 1 = Sim
            sfull = const.tile([128, FT, 2, BPC], f32, tag="sfull")

            # U/V accumulators (fp32), ping-pong chains: DVE on even kt,
            # GpSimd on odd kt; merged at the end.
            accs = {}
            for name in ("e0", "e1", "o0", "o1"):
                accs[name] = const.tile([128, 2, BPC * 64], f32,
                                        tag=f"acc_{name}",
                                        name=f"acc_{name}")
            nc.vector.memset(accs["e0"][:], 0.0)
            nc.gpsimd.memset(accs["o0"][:], 0.0)

            # ---- stage A+B: projections, products, segmented reduce ----
            for ft in range(FT):
                a_t = apool.tile([128, 4, 512], f16, tag="a")
                nc.sync.dma_start(
                    a_t[:], a_d[ft].rearrange("(ck p) m -> p ck m", p=128)
                )
                # two 2-plane PSUM super-tiles, plane stride 1024 (2 banks)
                ps1 = pspool.tile([128, 2, 1024], f32, tag="p1", name=f"ps1_{ft}")
                ps2 = pspool.tile([128, 2, 1024], f32, tag="p2", name=f"ps2_{ft}")
                for half, pst in ((0, ps1), (1, ps2)):
                    for pl in range(2):
                        m = half * 2 + pl
                        msl = slice(m * 128, (m + 1) * 128)
                        for c0, cn in ((0, 512), (512, T - 512)):
                            for ck in range(4):
                                nc.tensor.matmul(
                                    pst[:, pl, c0:c0 + cn],
                                    a_t[:, ck, msl],
                                    x_sb[:, ck, c0:c0 + cn],
                                    start=(ck == 0),
                                    stop=(ck == 3),
                                )
                # casts: c1 = [re1, im1]; c2x = [im2, re2, -im2]
                c1 = castp.tile([128, 2, T], f16, tag="c1", name=f"c1_{ft}")
                nc.scalar.activation(c1[:], ps1[:, :, 0:T], Act.Copy)
                c2x = castp.tile([128, 3, T], f16, tag="c2", name=f"c2_{ft}")
                nc.scalar.activation(c2x[:, 1:3, :], ps2[:, :, 0:T], Act.Copy)
                nc.scalar.activation(c2x[:, 0, :], ps2[:, 1, 0:T], Act.Copy,
                                     scale=-1.0)
                # products: G[g, pl, t]; g0 = [rere, -imim], g1 = [reim, imre]
                G = gpool.tile([128, 2, 2, T], f16, tag="G", name=f"G_{ft}")
                nc.vector.tensor_tensor(G[:, 0], c1[:], c2x[:, 1:3, :], op=mult)
                nc.vector.tensor_tensor(G[:, 1], c1[:], c2x[:, 0:2, :], op=mult)
                # fold the two planes of each group (bf16 2x), then one
                # segmented reduce over t per batch elem -> S[k, g, b]
                GS = gpool.tile([128, 2, T], f16, tag="GS", name=f"GS_{ft}")
                nc.vector.tensor_tensor(GS[:], G[:, :, 0, :], G[:, :, 1, :],
                                        op=add)
                nc.vector.reduce_sum(
                    out=sfull[:, ft],
                    in_=GS[:].rearrange("p g (b t) -> p g b t", b=BPC),
                    axis=mybir.AxisListType.X,
                )

                # ---- stage C for this kt: twiddle + accumulate ----
                kt = ft
                s_ap = sfull[:, kt]                     # [128, 2, 4] fp32
                s_b = s_ap[:, :, :, None].broadcast_to([128, 2, BPC, 64])
                cu_b = cu_sb[:, kt][:, :, None, :].broadcast_to([128, 2, BPC, 64])
                cv_b = cv_sb[:, kt][:, :, None, :].broadcast_to([128, 2, BPC, 64])
                W = uvpool.tile([128, 4, BPC * 64], f32, tag="W", name=f"W_{kt}")
                uvp = uvpool.tile([128, 2, BPC * 64], f32, tag="uvp",
                                  name=f"uvp_{kt}")
                w4 = W[:].rearrange("p c (b s) -> p c b s", s=64)
                nc.gpsimd.tensor_tensor(w4[:, 0:2], cu_b, s_b, op=mult)
                nc.gpsimd.tensor_tensor(w4[:, 2:4], cv_b, s_b, op=mult)
                # comps (0,2)=[u1,v1] + comps (1,3)=[u2',v2] -> [uu, vv]
                wv = W[:].rearrange("p (a b) n -> p b a n", a=2, b=2)
                nc.gpsimd.tensor_tensor(uvp[:], wv[:, 0], wv[:, 1], op=add)
                if kt % 2 == 0:
                    src = accs["e0"] if (kt // 2) % 2 == 0 else accs["e1"]
                    dst = accs["e1"] if (kt // 2) % 2 == 0 else accs["e0"]
                    nc.vector.tensor_tensor(dst[:], src[:], uvp[:], op=add)
                else:
                    src = accs["o0"] if (kt // 2) % 2 == 0 else accs["o1"]
                    dst = accs["o1"] if (kt // 2) % 2 == 0 else accs["o0"]
                    nc.gpsimd.tensor_tensor(dst[:], src[:], uvp[:], op=add)

            # final accumulators (after 33 kts: even chain ends in e-dst of
            # kt=32 -> (32//2)%2==0 -> e1; odd chain: kt=31 -> (31//2)%2==1 -> o0)
            uv32 = scr.tile([128, 2, BPC * 64], f32, tag="uv32")
            nc.vector.tensor_tensor(uv32[:], accs["e1"][:], accs["o0"][:], op=add)

            # ---- IFFT: 2 matmuls over k mod 128 ----
            # reuse the stage-A PSUM allocations (pool is exactly 8 banks)
            psy_t = pspool.tile([128, 2, 1024], f32, tag="p1", name="psy_t")
            psy = psy_t[:, 0, 0:BPC * 64]
            nc.tensor.matmul(psy, cosa_sb[:], uv32[:, 0, :],
                             start=True, stop=False)
            nc.tensor.matmul(psy, nsina_sb[:], uv32[:, 1, :],
                             start=False, stop=True)

            # ---- stage D: signed sqrt, per-batch l2 norm, store ----
            absy = scr.tile([128, BPC * 64], f32, tag="absy")
            nc.scalar.activation(absy[:], psy, Act.Abs)
            sqy = scr.tile([128, BPC * 64], f32, tag="sqy")
            nc.scalar.activation(sqy[:], absy[:], Act.Sqrt)
            sgn = scr.tile([128, BPC * 64], f32, tag="sgn")
            nc.scalar.activation(sgn[:], psy, Act.Sign)
            ys = scr.tile([128, BPC * 64], f32, tag="ys")
            nc.vector.tensor_mul(ys[:], sqy[:], sgn[:])

            psn_t = pspool.tile([128, 2, 1024], f32, tag="p2", name="psn_t")
            psn = psn_t[:, 0, 0:BPC * 64]
            nc.tensor.matmul(psn[0:1, :], ones_sb[:], absy[:],
                             start=True, stop=True)
            nsq = scr.tile([1, BPC], f32, tag="nsq")
            nc.vector.reduce_sum(
                out=nsq[:],
                in_=psn[0:1, :].rearrange("p (b s) -> p b s", b=BPC),
                axis=mybir.AxisListType.X,
            )
            nc.vector.tensor_scalar_max(nsq[:], nsq[:], 1e-10)
            sqn = scr.tile([1, BPC], f32, tag="sqn")
            nc.scalar.activation(sqn[:], nsq[:], Act.Sqrt)
            invn = scr.tile([1, BPC], f32, tag="invn")
            nc.vector.reciprocal(invn[:], sqn[:])

            psb_t = pspool.tile([128, 2, 1024], f32, tag="p1", name="psb_t")
            psb = psb_t[:, 1, 0:BPC * 64]
            nc.tensor.matmul(psb[:, 0:BPC], onesrow[0:1, :], invn[0:1, :],
                             start=True, stop=True)
            inv_b = psb[:, 0:BPC][:, :, None].broadcast_to([128, BPC, 64])
            fin = scr.tile([128, BPC * 64], f32, tag="fin")
            nc.vector.tensor_tensor(
                fin[:].rearrange("p (b s) -> p b s", b=BPC),
                ys[:].rearrange("p (b s) -> p b s", b=BPC),
                inv_b,
                op=mult,
            )
            for b in range(BPC):
                nc.sync.dma_start(
                    y_d[b].rearrange("(q s) -> q s", q=128),
                    fin[:, b * 64:(b + 1) * 64],
                )

    nc.compile()
    return nc


def _host_prep(x, M1, M2):
    x = np.ascontiguousarray(np.asarray(x, np.float32))
    M1 = np.asarray(M1, np.float32)
    M2 = np.asarray(M2, np.float32)

    h1 = np.argmax(np.abs(M1), axis=1)
    s1 = M1[np.arange(C), h1].astype(np.float64)
    h2 = np.argmax(np.abs(M2), axis=1)
    s2 = M2[np.arange(C), h2].astype(np.float64)

    NSLOT = FT * 128
    k = np.arange(NSLOT, dtype=np.float64)
    valid = k <= P // 2
    ang1 = 2 * np.pi * np.outer(h1.astype(np.float64), k) / P
    ang2 = 2 * np.pi * np.outer(h2.astype(np.float64), k) / P
    # a[ft, c, m*128 + j]: m in (A1re, A1im, A2re, -A2im), freq = ft*128 + j
    a = np.empty((FT, C, 512), np.float32)
    a1re = (s1[:, None] * np.cos(ang1) * valid).astype(np.float32)
    a1im = (-s1[:, None] * np.sin(ang1) * valid).astype(np.float32)
    a2re = (s2[:, None] * np.cos(ang2) * valid).astype(np.float32)
    a2imn = (s2[:, None] * np.sin(ang2) * valid).astype(np.float32)  # -A2im
    for ft in range(FT):
        ksl = slice(ft * 128, (ft + 1) * 128)
        a[ft, :, 0:128] = a1re[:, ksl]
        a[ft, :, 128:256] = a1im[:, ksl]
        a[ft, :, 256:384] = a2re[:, ksl]
        a[ft, :, 384:512] = a2imn[:, ksl]

    w = np.where(valid, 2.0 / P, 0.0)
    w[0] = 1.0 / P
    w[P // 2] = 1.0 / P
    s_idx = np.arange(64, dtype=np.float64)
    phi = 2 * np.pi * np.outer(k, s_idx) / P
    cphi = (w[:, None] * np.cos(phi)).astype(np.float32).reshape(FT, 128, 64)
    sphi = (w[:, None] * np.sin(phi)).astype(np.float32).reshape(FT, 128, 64)
    # cu = [cphi, -sphi] (U = cphi*Sre - sphi*Sim), cv = [sphi, cphi]
    cu = np.stack([cphi, -sphi], axis=2)   # [FT, 128, 2, 64]
    cv = np.stack([sphi, cphi], axis=2)

    km = np.arange(128, dtype=np.float64)
    alpha = 2 * np.pi * np.outer(km, km) / 128
    cosa = np.cos(alpha).astype(np.float32)
    nsina = (-np.sin(alpha)).astype(np.float32)

    xt = np.ascontiguousarray(x.reshape(B * HW, C).T)  # [C, 6272]

    return (a.astype(np.float16), cu, cv, cosa, nsina, xt.astype(np.float16))


def _make_in_maps(x, M1, M2):
    a, cu, cv, cosa, nsina, xt = _host_prep(x, M1, M2)
    in_maps = []
    for r in range(NCORES):
        in_maps.append({
            "a": a,
            "x": np.ascontiguousarray(xt[:, r * T:(r + 1) * T]),
            "cu": cu,
            "cv": cv,
            "cosa": cosa,
            "nsina": nsina,
        })
    return in_maps


def kernel(x, M1, M2):
    from concourse.bass_utils import run_bass_kernel_spmd

    if "nc" not in _CACHE:
        _CACHE["nc"] = _build_program()
    nc = _CACHE["nc"]

    in_maps = _make_in_maps(x, M1, M2)
    res = run_bass_kernel_spmd(nc, in_maps, core_ids=list(range(NCORES)))
    out = np.concatenate([res.results[r]["y"] for r in range(NCORES)], axis=0)
    return out.astype(np.float32)


# revision 13
# speedup vs baseline: 2.4445x; 1.1102x over previous
"""Trainium2 kernel for CompactBilinearLayer (count-sketch bilinear pooling).

Math: y = l2norm(signed_sqrt(sum_hw Re IFFT(FFT(x@M1)*FFT(x@M2)))).
FFT(x@M1) == x @ A1 with A1[c,k] = s1[c] exp(-2pi i h1[c] k/P) (dense [C,K],
host-built).  IFFT is linear so the spatial sum moves before it; Hermitian
symmetry keeps only k = 0..4096 (padded to 33*128 slots).

Per core (4 batch elems, T=784 spatial positions, no collectives):
  A: P-planes = A^T @ x^T in bf16 (tolerance 2e-2 >> bf16 error) as two
     2-plane PSUM super-tiles (re1,im1) and (re2,-im2).
  B: casts to bf16 SBUF (with an extra negated im2 plane so both complex
     product groups are pure ADDs), pair-packed DVE products, bf16 pair-fold,
     one segmented reduce -> S[k, b] (re, im).
  C: per kt twiddle U=cphi*Sre-sphi*Sim, V=sphi*Sre+cphi*Sim as packed
     TTs (GpSimd+DVE), accumulated over kt into Utot/Vtot; since the DFT-128
     matrix depends only on k mod 128, IFFT = 2 matmuls at the end.
  D: signed sqrt + per-batch L2 norm + store.
"""
import numpy as np

P = 8192
C = 512
FT = 33            # frequency tiles of 128 -> 4224 slots >= 4097
NCORES = 8
BPC = 4            # batch elems per core
HW = 196           # spatial positions per batch elem
T = BPC * HW       # 784 positions per core
B = 32

_CACHE = {}


def _build_program():
    import concourse.bass as bass
    import concourse.tile as tile
    from concourse import bacc, mybir

    f32 = mybir.dt.float32
    f16 = mybir.dt.float16
    nc = bacc.Bacc("TRN2", target_bir_lowering=False, debug=False,
                   num_devices=NCORES)

    a_d = nc.dram_tensor("a", [FT, C, 512], f16, kind="ExternalInput").ap()
    x_d = nc.dram_tensor("x", [C, T], f16, kind="ExternalInput").ap()
    cucv_d = nc.dram_tensor("cucv", [FT, 128, 4, 64], f32,
                            kind="ExternalInput").ap()
    cosa_d = nc.dram_tensor("cosa", [128, 128], f32, kind="ExternalInput").ap()
    nsina_d = nc.dram_tensor("nsina", [128, 128], f32, kind="ExternalInput").ap()
    y_d = nc.dram_tensor("y", [BPC, P], f32, kind="ExternalOutput").ap()

    mult = mybir.AluOpType.mult
    add = mybir.AluOpType.add
    Act = mybir.ActivationFunctionType

    with tile.TileContext(nc) as tc:
        with (
            tc.tile_pool(name="const", bufs=1) as const,
            tc.tile_pool(name="apool", bufs=3) as apool,
            tc.tile_pool(name="ps", bufs=1, space="PSUM") as pspool,
            tc.tile_pool(name="cast", bufs=2) as castp,
            tc.tile_pool(name="gp", bufs=2) as gpool,
            tc.tile_pool(name="uv", bufs=3) as uvpool,
            tc.tile_pool(name="sf", bufs=3) as sfpool,
            tc.tile_pool(name="scr", bufs=2) as scr,
        ):
            x_sb = const.tile([128, 4, T], f16)
            nc.sync.dma_start(x_sb[:], x_d.rearrange("(ck p) t -> p ck t", p=128))
            cucv_sb = const.tile([128, FT, 4, 64], f32)
            nc.sync.dma_start(cucv_sb[:], cucv_d.rearrange("kt p c s -> p kt c s"))
            cosa_sb = const.tile([128, 128], f32)
            nc.sync.dma_start(cosa_sb[:], cosa_d)
            nsina_sb = const.tile([128, 128], f32)
            nc.sync.dma_start(nsina_sb[:], nsina_d)
            ones_sb = const.tile([128, 1], f32)
            nc.vector.memset(ones_sb[:], 1.0)
            onesrow = const.tile([1, 128], f32)
            nc.vector.memset(onesrow[:], 1.0)



            # U/V accumulators (fp32), ping-pong chains: DVE on even kt,
            # GpSimd on odd kt; merged at the end.
            accs = {}
            for name in ("e0", "e1"):
                accs[name] = const.tile([128, 2, BPC * 64], f32,
                                        tag=f"acc_{name}",
                                        name=f"acc_{name}")
            nc.gpsimd.memset(accs["e0"][:], 0.0)

            # ---- stage A+B: projections, products, segmented reduce ----
            for ft in range(FT):
                a_t = apool.tile([128, 4, 512], f16, tag="a")
                nc.sync.dma_start(
                    a_t[:], a_d[ft].rearrange("(ck p) m -> p ck m", p=128)
                )
                # two 2-plane PSUM super-tiles, plane stride 1024 (2 banks)
                ps1 = pspool.tile([128, 2, 1024], f32, tag="p1", name=f"ps1_{ft}")
                ps2 = pspool.tile([128, 2, 1024], f32, tag="p2", name=f"ps2_{ft}")
                for half, pst in ((0, ps1), (1, ps2)):
                    for pl in range(2):
                        m = half * 2 + pl
                        msl = slice(m * 128, (m + 1) * 128)
                        for c0, cn in ((0, 512), (512, T - 512)):
                            for ck in range(4):
                                nc.tensor.matmul(
                                    pst[:, pl, c0:c0 + cn],
                                    a_t[:, ck, msl],
                                    x_sb[:, ck, c0:c0 + cn],
                                    start=(ck == 0),
                                    stop=(ck == 3),
                                )
                # casts: c1 = [re1, im1]; c2x = [im2, re2, -im2]
                c1 = castp.tile([128, 2, T], f16, tag="c1", name=f"c1_{ft}")
                nc.scalar.activation(c1[:], ps1[:, :, 0:T], Act.Copy)
                c2x = castp.tile([128, 3, T], f16, tag="c2", name=f"c2_{ft}")
                nc.scalar.activation(c2x[:, 1:3, :], ps2[:, :, 0:T], Act.Copy)
                nc.scalar.activation(c2x[:, 0, :], ps2[:, 1, 0:T], Act.Copy,
                                     scale=-1.0)
                # products: G[g, pl, t]; g0 = [rere, -imim], g1 = [reim, imre]
                G = gpool.tile([128, 2, 2, T], f16, tag="G", name=f"G_{ft}")
                nc.vector.tensor_tensor(G[:, 0], c1[:], c2x[:, 1:3, :], op=mult)
                nc.vector.tensor_tensor(G[:, 1], c1[:], c2x[:, 0:2, :], op=mult)
                # fold the two planes of each group (bf16 2x), then one
                # segmented reduce over t per batch elem -> S[k, g, b]
                GS = gpool.tile([128, 2, T], f16, tag="GS", name=f"GS_{ft}")
                nc.vector.tensor_tensor(GS[:], G[:, :, 0, :], G[:, :, 1, :],
                                        op=add)
                sf = sfpool.tile([128, 2, BPC], f32, tag="sf", name=f"sf_{ft}")
                nc.vector.reduce_sum(
                    out=sf[:],
                    in_=GS[:].rearrange("p g (b t) -> p g b t", b=BPC),
                    axis=mybir.AxisListType.X,
                )

                # ---- stage C for this kt: twiddle + accumulate ----
                # W comps = [u1, u2n, v1, v2] = cucv(kt) * S[g-pattern 0,1,0,1]
                kt = ft
                s_b = sf[:, :, :, None].broadcast_to([128, 2, BPC, 64])
                cu_b = cucv_sb[:, kt, 0:2][:, :, None, :].broadcast_to(
                    [128, 2, BPC, 64])
                cv_b = cucv_sb[:, kt, 2:4][:, :, None, :].broadcast_to(
                    [128, 2, BPC, 64])
                W = uvpool.tile([128, 4, BPC * 64], f32, tag="W", name=f"W_{kt}")
                uvp = uvpool.tile([128, 2, BPC * 64], f32, tag="uvp",
                                  name=f"uvp_{kt}")
                w4 = W[:].rearrange("p c (b s) -> p c b s", s=64)
                nc.gpsimd.tensor_tensor(w4[:, 0:2], cu_b, s_b, op=mult)
                nc.gpsimd.tensor_tensor(w4[:, 2:4], cv_b, s_b, op=mult)
                # comps (0,2)=[u1,v1] + comps (1,3)=[u2n,v2] -> [uu, vv]
                wv = W[:].rearrange("p (a b) n -> p b a n", a=2, b=2)
                nc.gpsimd.tensor_tensor(uvp[:], wv[:, 0], wv[:, 1], op=add)
                acc_src = accs["e0"] if kt % 2 == 0 else accs["e1"]
                acc_dst = accs["e1"] if kt % 2 == 0 else accs["e0"]
                nc.gpsimd.tensor_tensor(acc_dst[:], acc_src[:], uvp[:], op=add)

            uv32 = accs["e1"]

            # ---- IFFT: 2 matmuls over k mod 128 ----
            # reuse the stage-A PSUM allocations (pool is exactly 8 banks)
            psy_t = pspool.tile([128, 2, 1024], f32, tag="p1", name="psy_t")
            psy = psy_t[:, 0, 0:BPC * 64]
            nc.tensor.matmul(psy, cosa_sb[:], uv32[:, 0, :],
                             start=True, stop=False)
            nc.tensor.matmul(psy, nsina_sb[:], uv32[:, 1, :],
                             start=False, stop=True)

            # ---- stage D: signed sqrt, per-batch l2 norm, store ----
            absy = scr.tile([128, BPC * 64], f32, tag="absy")
            nc.scalar.activation(absy[:], psy, Act.Abs)
            sqy = scr.tile([128, BPC * 64], f32, tag="sqy")
            nc.scalar.activation(sqy[:], absy[:], Act.Sqrt)
            sgn = scr.tile([128, BPC * 64], f32, tag="sgn")
            nc.scalar.activation(sgn[:], psy, Act.Sign)
            ys = scr.tile([128, BPC * 64], f32, tag="ys")
            nc.vector.tensor_mul(ys[:], sqy[:], sgn[:])

            psn_t = pspool.tile([128, 2, 1024], f32, tag="p2", name="psn_t")
            psn = psn_t[:, 0, 0:BPC * 64]
            nc.tensor.matmul(psn[0:1, :], ones_sb[:], absy[:],
                             start=True, stop=True)
            nsq = scr.tile([1, BPC], f32, tag="nsq")
            nc.vector.reduce_sum(
                out=nsq[:],
                in_=psn[0:1, :].rearrange("p (b s) -> p b s", b=BPC),
                axis=mybir.AxisListType.X,
            )
            nc.vector.tensor_scalar_max(nsq[:], nsq[:], 1e-10)
            sqn = scr.tile([1, BPC], f32, tag="sqn")
            nc.scalar.activation(sqn[:], nsq[:], Act.Sqrt)
            invn = scr.tile([1, BPC], f32, tag="invn")
            nc.vector.reciprocal(invn[:], sqn[:])

            psb_t = pspool.tile([128, 2, 1024], f32, tag="p1", name="psb_t")
            psb = psb_t[:, 1, 0:BPC * 64]
            nc.tensor.matmul(psb[:, 0:BPC], onesrow[0:1, :], invn[0:1, :],
                             start=True, stop=True)
            inv_b = psb[:, 0:BPC][:, :, None].broadcast_to([128, BPC, 64])
            fin = scr.tile([128, BPC * 64], f32, tag="fin")
            nc.vector.tensor_tensor(
                fin[:].rearrange("p (b s) -> p b s", b=BPC),
                ys[:].rearrange("p (b s) -> p b s", b=BPC),
                inv_b,
                op=mult,
            )
            for b in range(BPC):
                nc.sync.dma_start(
                    y_d[b].rearrange("(q s) -> q s", q=128),
                    fin[:, b * 64:(b + 1) * 64],
                )

    nc.compile()
    return nc


def _host_prep(x, M1, M2):
    x = np.ascontiguousarray(np.asarray(x, np.float32))
    M1 = np.asarray(M1, np.float32)
    M2 = np.asarray(M2, np.float32)

    h1 = np.argmax(np.abs(M1), axis=1)
    s1 = M1[np.arange(C), h1].astype(np.float64)
    h2 = np.argmax(np.abs(M2), axis=1)
    s2 = M2[np.arange(C), h2].astype(np.float64)

    NSLOT = FT * 128
    k = np.arange(NSLOT, dtype=np.float64)
    valid = k <= P // 2
    ang1 = 2 * np.pi * np.outer(h1.astype(np.float64), k) / P
    ang2 = 2 * np.pi * np.outer(h2.astype(np.float64), k) / P
    # a[ft, c, m*128 + j]: m in (A1re, A1im, A2re, -A2im), freq = ft*128 + j
    a = np.empty((FT, C, 512), np.float32)
    a1re = (s1[:, None] * np.cos(ang1) * valid).astype(np.float32)
    a1im = (-s1[:, None] * np.sin(ang1) * valid).astype(np.float32)
    a2re = (s2[:, None] * np.cos(ang2) * valid).astype(np.float32)
    a2imn = (s2[:, None] * np.sin(ang2) * valid).astype(np.float32)  # -A2im
    for ft in range(FT):
        ksl = slice(ft * 128, (ft + 1) * 128)
        a[ft, :, 0:128] = a1re[:, ksl]
        a[ft, :, 128:256] = a1im[:, ksl]
        a[ft, :, 256:384] = a2re[:, ksl]
        a[ft, :, 384:512] = a2imn[:, ksl]

    w = np.where(valid, 2.0 / P, 0.0)
    w[0] = 1.0 / P
    w[P // 2] = 1.0 / P
    s_idx = np.arange(64, dtype=np.float64)
    phi = 2 * np.pi * np.outer(k, s_idx) / P
    cphi = (w[:, None] * np.cos(phi)).astype(np.float32).reshape(FT, 128, 64)
    sphi = (w[:, None] * np.sin(phi)).astype(np.float32).reshape(FT, 128, 64)
    # cucv comps: [cphi, -sphi, sphi, cphi] so W = [u1, u2n, v1, v2]
    cucv = np.stack([cphi, -sphi, sphi, cphi], axis=2)  # [FT, 128, 4, 64]

    km = np.arange(128, dtype=np.float64)
    alpha = 2 * np.pi * np.outer(km, km) / 128
    cosa = np.cos(alpha).astype(np.float32)
    nsina = (-np.sin(alpha)).astype(np.float32)

    xt = np.ascontiguousarray(x.reshape(B * HW, C).T)  # [C, 6272]

    return (a.astype(np.float16), cucv, cosa, nsina, xt.astype(np.float16))


def _make_in_maps(x, M1, M2):
    a, cucv, cosa, nsina, xt = _host_prep(x, M1, M2)
    in_maps = []
    for r in range(NCORES):
        in_maps.append({
            "a": a,
            "x": np.ascontiguousarray(xt[:, r * T:(r + 1) * T]),
            "cucv": cucv,
            "cosa": cosa,
            "nsina": nsina,
        })
    return in_maps


def kernel(x, M1, M2):
    from concourse.bass_utils import run_bass_kernel_spmd

    if "nc" not in _CACHE:
        _CACHE["nc"] = _build_program()
    nc = _CACHE["nc"]

    in_maps = _make_in_maps(x, M1, M2)
    res = run_bass_kernel_spmd(nc, in_maps, core_ids=list(range(NCORES)))
    out = np.concatenate([res.results[r]["y"] for r in range(NCORES)], axis=0)
    return out.astype(np.float32)


# revision 14
# speedup vs baseline: 2.6801x; 1.0964x over previous
"""Trainium2 kernel for CompactBilinearLayer (count-sketch bilinear pooling).

Math: y = l2norm(signed_sqrt(sum_hw Re IFFT(FFT(x@M1)*FFT(x@M2)))).
FFT(x@M1) == x @ A1 with A1[c,k] = s1[c] exp(-2pi i h1[c] k/P) (dense [C,K],
host-built).  IFFT is linear so the spatial sum moves before it; Hermitian
symmetry keeps only k = 0..4096 (padded to 33*128 slots).

Per core (4 batch elems, T=784 spatial positions, no collectives):
  A: P-planes = A^T @ x^T in bf16 (tolerance 2e-2 >> bf16 error) as two
     2-plane PSUM super-tiles (re1,im1) and (re2,-im2).
  B: casts to bf16 SBUF (with an extra negated im2 plane so both complex
     product groups are pure ADDs), pair-packed DVE products, bf16 pair-fold,
     one segmented reduce -> S[k, b] (re, im).
  C: per kt twiddle U=cphi*Sre-sphi*Sim, V=sphi*Sre+cphi*Sim as packed
     TTs (GpSimd+DVE), accumulated over kt into Utot/Vtot; since the DFT-128
     matrix depends only on k mod 128, IFFT = 2 matmuls at the end.
  D: signed sqrt + per-batch L2 norm + store.
"""
import numpy as np

P = 8192
C = 512
FT = 33            # frequency tiles of 128 -> 4224 slots >= 4097
NCORES = 8
BPC = 4            # batch elems per core
HW = 196           # spatial positions per batch elem
T = BPC * HW       # 784 positions per core
B = 32

_CACHE = {}


def _build_program():
    import concourse.bass as bass
    import concourse.tile as tile
    from concourse import bacc, mybir

    f32 = mybir.dt.float32
    f16 = mybir.dt.float16
    nc = bacc.Bacc("TRN2", target_bir_lowering=False, debug=False,
                   num_devices=NCORES)

    a_d = nc.dram_tensor("a", [FT, C, 512], f16, kind="ExternalInput").ap()
    x_d = nc.dram_tensor("x", [C, T], f16, kind="ExternalInput").ap()
    cucv_d = nc.dram_tensor("cucv", [FT, 128, 4, 64], f16,
                            kind="ExternalInput").ap()
    cosa_d = nc.dram_tensor("cosa", [128, 128], f16, kind="ExternalInput").ap()
    nsina_d = nc.dram_tensor("nsina", [128, 128], f16, kind="ExternalInput").ap()
    y_d = nc.dram_tensor("y", [BPC, P], f32, kind="ExternalOutput").ap()

    mult = mybir.AluOpType.mult
    add = mybir.AluOpType.add
    Act = mybir.ActivationFunctionType

    with tile.TileContext(nc) as tc:
        with (
            tc.tile_pool(name="const", bufs=1) as const,
            tc.tile_pool(name="apool", bufs=3) as apool,
            tc.tile_pool(name="ps", bufs=1, space="PSUM") as pspool,
            tc.tile_pool(name="cast", bufs=2) as castp,
            tc.tile_pool(name="gp", bufs=2) as gpool,
            tc.tile_pool(name="uv", bufs=3) as uvpool,
            tc.tile_pool(name="sf", bufs=3) as sfpool,
            tc.tile_pool(name="scr", bufs=2) as scr,
        ):
            x_sb = const.tile([128, 4, T], f16)
            nc.sync.dma_start(x_sb[:], x_d.rearrange("(ck p) t -> p ck t", p=128))
            a_pre = {}
            for ft in (0, 1):
                a_pre[ft] = apool.tile([128, 4, 512], f16, tag="a",
                                       name=f"a_pre{ft}")
                nc.sync.dma_start(
                    a_pre[ft][:], a_d[ft].rearrange("(ck p) m -> p ck m", p=128)
                )
            cucv_sb = const.tile([128, FT, 4, 64], f16)
            nc.sync.dma_start(cucv_sb[:], cucv_d.rearrange("kt p c s -> p kt c s"))
            cosa_sb = const.tile([128, 128], f16)
            nc.sync.dma_start(cosa_sb[:], cosa_d)
            nsina_sb = const.tile([128, 128], f16)
            nc.sync.dma_start(nsina_sb[:], nsina_d)
            ones_sb = const.tile([128, 1], f32)
            nc.vector.memset(ones_sb[:], 1.0)
            onesrow = const.tile([1, 128], f32)
            nc.vector.memset(onesrow[:], 1.0)



            # U/V accumulators (fp32), ping-pong chains: DVE on even kt,
            # GpSimd on odd kt; merged at the end.
            accs = {}
            for name in ("e0", "e1"):
                accs[name] = const.tile([128, 4, BPC * 64], f16,
                                        tag=f"acc_{name}",
                                        name=f"acc_{name}")
            nc.gpsimd.memset(accs["e0"][:], 0.0)

            # ---- stage A+B: projections, products, segmented reduce ----
            for ft in range(FT):
                if ft in a_pre:
                    a_t = a_pre.pop(ft)
                else:
                    a_t = apool.tile([128, 4, 512], f16, tag="a",
                                     name=f"a_{ft}")
                    nc.sync.dma_start(
                        a_t[:], a_d[ft].rearrange("(ck p) m -> p ck m", p=128)
                    )
                # two 2-plane PSUM super-tiles, plane stride 1024 (2 banks)
                ps1 = pspool.tile([128, 2, 1024], f32, tag="p1", name=f"ps1_{ft}")
                ps2 = pspool.tile([128, 2, 1024], f32, tag="p2", name=f"ps2_{ft}")
                for half, pst in ((0, ps1), (1, ps2)):
                    for pl in range(2):
                        m = half * 2 + pl
                        msl = slice(m * 128, (m + 1) * 128)
                        for c0, cn in ((0, 512), (512, T - 512)):
                            for ck in range(4):
                                nc.tensor.matmul(
                                    pst[:, pl, c0:c0 + cn],
                                    a_t[:, ck, msl],
                                    x_sb[:, ck, c0:c0 + cn],
                                    start=(ck == 0),
                                    stop=(ck == 3),
                                )
                # casts: c1 = [re1, im1]; c2x = [im2, re2, -im2]
                c1 = castp.tile([128, 2, T], f16, tag="c1", name=f"c1_{ft}")
                nc.scalar.activation(c1[:], ps1[:, :, 0:T], Act.Copy)
                c2x = castp.tile([128, 3, T], f16, tag="c2", name=f"c2_{ft}")
                nc.scalar.activation(c2x[:, 1:3, :], ps2[:, :, 0:T], Act.Copy)
                nc.scalar.activation(c2x[:, 0, :], ps2[:, 1, 0:T], Act.Copy,
                                     scale=-1.0)
                # products: G[g, pl, t]; g0 = [rere, -imim], g1 = [reim, imre]
                G = gpool.tile([128, 2, 2, T], f16, tag="G", name=f"G_{ft}")
                nc.vector.tensor_tensor(G[:, 0], c1[:], c2x[:, 1:3, :], op=mult)
                nc.vector.tensor_tensor(G[:, 1], c1[:], c2x[:, 0:2, :], op=mult)
                # fold the two planes of each group (bf16 2x), then one
                # segmented reduce over t per batch elem -> S[k, g, b]
                GS = gpool.tile([128, 2, T], f16, tag="GS", name=f"GS_{ft}")
                nc.vector.tensor_tensor(GS[:], G[:, :, 0, :], G[:, :, 1, :],
                                        op=add)
                sf = sfpool.tile([128, 2, BPC], f32, tag="sf", name=f"sf_{ft}")
                nc.vector.reduce_sum(
                    out=sf[:],
                    in_=GS[:].rearrange("p g (b t) -> p g b t", b=BPC),
                    axis=mybir.AxisListType.X,
                )

                # ---- stage C for this kt: twiddle + accumulate ----
                # W comps = [u1, u2n, v1, v2] = cucv(kt) * S[g-pattern 0,1,0,1]
                kt = ft
                s_b = sf[:, :, :, None].broadcast_to([128, 2, BPC, 64])
                cu_b = cucv_sb[:, kt, 0:2][:, :, None, :].broadcast_to(
                    [128, 2, BPC, 64])
                cv_b = cucv_sb[:, kt, 2:4][:, :, None, :].broadcast_to(
                    [128, 2, BPC, 64])
                W = uvpool.tile([128, 4, BPC * 64], f16, tag="W", name=f"W_{kt}")
                w4 = W[:].rearrange("p c (b s) -> p c b s", s=64)
                nc.gpsimd.tensor_tensor(w4[:, 0:2], cu_b, s_b, op=mult)
                nc.gpsimd.tensor_tensor(w4[:, 2:4], cv_b, s_b, op=mult)
                acc_src = accs["e0"] if kt % 2 == 0 else accs["e1"]
                acc_dst = accs["e1"] if kt % 2 == 0 else accs["e0"]
                nc.gpsimd.tensor_tensor(acc_dst[:], acc_src[:], W[:], op=add)

            acc = accs["e1"]

            # ---- IFFT over k mod 128: psy = cosa@(u1+u2n) + nsina@(v1+v2)
            # reuse the stage-A PSUM allocations (pool is exactly 8 banks)
            psy_t = pspool.tile([128, 2, 1024], f32, tag="p1", name="psy_t")
            psy = psy_t[:, 0, 0:BPC * 64]
            nc.tensor.matmul(psy, cosa_sb[:], acc[:, 0, :],
                             start=True, stop=False)
            nc.tensor.matmul(psy, cosa_sb[:], acc[:, 1, :],
                             start=False, stop=False)
            nc.tensor.matmul(psy, nsina_sb[:], acc[:, 2, :],
                             start=False, stop=False)
            nc.tensor.matmul(psy, nsina_sb[:], acc[:, 3, :],
                             start=False, stop=True)

            # ---- stage D: signed sqrt, per-batch l2 norm, store ----
            absy = scr.tile([128, BPC * 64], f32, tag="absy")
            nc.scalar.activation(absy[:], psy, Act.Abs)
            sqy = scr.tile([128, BPC * 64], f32, tag="sqy")
            nc.scalar.activation(sqy[:], absy[:], Act.Sqrt)
            sgn = scr.tile([128, BPC * 64], f32, tag="sgn")
            nc.scalar.activation(sgn[:], psy, Act.Sign)
            ys = scr.tile([128, BPC * 64], f32, tag="ys")
            nc.vector.tensor_mul(ys[:], sqy[:], sgn[:])

            psn_t = pspool.tile([128, 2, 1024], f32, tag="p2", name="psn_t")
            psn = psn_t[:, 0, 0:BPC * 64]
            nc.tensor.matmul(psn[0:1, :], ones_sb[:], absy[:],
                             start=True, stop=True)
            nsq = scr.tile([1, BPC], f32, tag="nsq")
            nc.vector.reduce_sum(
                out=nsq[:],
                in_=psn[0:1, :].rearrange("p (b s) -> p b s", b=BPC),
                axis=mybir.AxisListType.X,
            )
            nc.vector.tensor_scalar_max(nsq[:], nsq[:], 1e-10)
            sqn = scr.tile([1, BPC], f32, tag="sqn")
            nc.scalar.activation(sqn[:], nsq[:], Act.Sqrt)
            invn = scr.tile([1, BPC], f32, tag="invn")
            nc.vector.reciprocal(invn[:], sqn[:])

            psb_t = pspool.tile([128, 2, 1024], f32, tag="p1", name="psb_t")
            psb = psb_t[:, 1, 0:BPC * 64]
            nc.tensor.matmul(psb[:, 0:BPC], onesrow[0:1, :], invn[0:1, :],
                             start=True, stop=True)
            inv_b = psb[:, 0:BPC][:, :, None].broadcast_to([128, BPC, 64])
            fin = scr.tile([128, BPC * 64], f32, tag="fin")
            nc.vector.tensor_tensor(
                fin[:].rearrange("p (b s) -> p b s", b=BPC),
                ys[:].rearrange("p (b s) -> p b s", b=BPC),
                inv_b,
                op=mult,
            )
            for b in range(BPC):
                nc.sync.dma_start(
                    y_d[b].rearrange("(q s) -> q s", q=128),
                    fin[:, b * 64:(b + 1) * 64],
                )

    nc.compile()
    return nc


def _host_prep(x, M1, M2):
    x = np.ascontiguousarray(np.asarray(x, np.float32))
    M1 = np.asarray(M1, np.float32)
    M2 = np.asarray(M2, np.float32)

    h1 = np.argmax(np.abs(M1), axis=1)
    s1 = M1[np.arange(C), h1].astype(np.float64)
    h2 = np.argmax(np.abs(M2), axis=1)
    s2 = M2[np.arange(C), h2].astype(np.float64)

    NSLOT = FT * 128
    k = np.arange(NSLOT, dtype=np.float64)
    valid = k <= P // 2
    ang1 = 2 * np.pi * np.outer(h1.astype(np.float64), k) / P
    ang2 = 2 * np.pi * np.outer(h2.astype(np.float64), k) / P
    # a[ft, c, m*128 + j]: m in (A1re, A1im, A2re, -A2im), freq = ft*128 + j
    a = np.empty((FT, C, 512), np.float32)
    a1re = (s1[:, None] * np.cos(ang1) * valid).astype(np.float32)
    a1im = (-s1[:, None] * np.sin(ang1) * valid).astype(np.float32)
    a2re = (s2[:, None] * np.cos(ang2) * valid).astype(np.float32)
    a2imn = (s2[:, None] * np.sin(ang2) * valid).astype(np.float32)  # -A2im
    for ft in range(FT):
        ksl = slice(ft * 128, (ft + 1) * 128)
        a[ft, :, 0:128] = a1re[:, ksl]
        a[ft, :, 128:256] = a1im[:, ksl]
        a[ft, :, 256:384] = a2re[:, ksl]
        a[ft, :, 384:512] = a2imn[:, ksl]

    w = np.where(valid, 2.0 / P, 0.0)
    w[0] = 1.0 / P
    w[P // 2] = 1.0 / P
    s_idx = np.arange(64, dtype=np.float64)
    phi = 2 * np.pi * np.outer(k, s_idx) / P
    cphi = (w[:, None] * np.cos(phi)).astype(np.float32).reshape(FT, 128, 64)
    sphi = (w[:, None] * np.sin(phi)).astype(np.float32).reshape(FT, 128, 64)
    # cucv comps: [cphi, -sphi, sphi, cphi] so W = [u1, u2n, v1, v2]
    cucv = np.stack([cphi, -sphi, sphi, cphi], axis=2)  # [FT, 128, 4, 64]

    km = np.arange(128, dtype=np.float64)
    alpha = 2 * np.pi * np.outer(km, km) / 128
    cosa = np.cos(alpha).astype(np.float32)
    nsina = (-np.sin(alpha)).astype(np.float32)

    xt = np.ascontiguousarray(x.reshape(B * HW, C).T)  # [C, 6272]

    return (a.astype(np.float16), cucv.astype(np.float16), cosa.astype(np.float16),
            nsina.astype(np.float16), xt.astype(np.float16))


def _make_in_maps(x, M1, M2):
    a, cucv, cosa, nsina, xt = _host_prep(x, M1, M2)
    in_maps = []
    for r in range(NCORES):
        in_maps.append({
            "a": a,
            "x": np.ascontiguousarray(xt[:, r * T:(r + 1) * T]),
            "cucv": cucv,
            "cosa": cosa,
            "nsina": nsina,
        })
    return in_maps


def kernel(x, M1, M2):
    from concourse.bass_utils import run_bass_kernel_spmd

    if "nc" not in _CACHE:
        _CACHE["nc"] = _build_program()
    nc = _CACHE["nc"]

    in_maps = _make_in_maps(x, M1, M2)
    res = run_bass_kernel_spmd(nc, in_maps, core_ids=list(range(NCORES)))
    out = np.concatenate([res.results[r]["y"] for r in range(NCORES)], axis=0)
    return out.astype(np.float32)
